# revision 11
# baseline (speedup 1.0000x reference)
"""Trainium2 Bass kernel for YOLO-style DetectionLayer decode.

Full input  x: (16, 255, 76, 76) f32  (channel-major: 3 anchors x 85 ch)
Full output  : (16, 17328, 85) f32   (position-major: 3*76*76 rows x 85 ch)

Math per (b, a, gy, gx):
  out[..., 0] = (sigmoid(tx) + gx) * 8
  out[..., 1] = (sigmoid(ty) + gy) * 8
  out[..., 2] = exp(tw) * ANCHOR[a][0]        (stride cancels)
  out[..., 3] = exp(th) * ANCHOR[a][1]
  out[..., 4:] = sigmoid(...)

Sharding: pure data-parallel over batch: 2 batches per core x 8 cores.

Per-core kernel (per (b, a) pair, 6 pairs):
  - Input block (85 ch, 5776 pos) loads with channels on partitions.
    SBUF ports interleave mod 64 (port 2j serves partitions {4j..4j+3,
    32+4j..35+4j}; odd ports the same at +64), so a fixed 0..84 layout
    piles 8 DMA packets on every even port and <=4 on odd ports.  Pairs
    alternate between partitions 0..84 and 43..127 so overlapping input
    DMAs of adjacent pairs hit complementary ports (max 6/pair).
  - TensorE transposes 46 chunks of (128 part, 128 pos) via one-hot
    selector matrices (I85 at rows 0..84 / rows 43..127; junk rows hit
    zero columns) -> PSUM (128 pos, 85 ch).  Chunk j takes positions
    {45 p + j} so output partition p holds 45 consecutive output rows
    -> 15.3KB contiguous output DMA runs.
  - ScalarE evacuates PSUM with fused tanh(v/2) (sigmoid = .5+.5*tanh;
    one ACT table set has both tanh and exp), plus true Exp on the w/h
    cols straight from PSUM raw values.
  - VectorE: whole-tile affine .5*t+.5 (2x port mode) turns tanh into
    sigmoid; x/y = 8*s + 8*grid (host table); w/h = (2A)*v - A
    (compensates the affine on the exp'd cols).
  - Output DMA per pair on the GpSimd SWDGE queue (sync HWDGE carries
    the input loads; separate rings, and GpSimd is otherwise idle).
"""

import os
import sys

import numpy as np

for _p in ("/opt/trn_rl_repo", "/root/.axon_site/_ro/trn_rl_repo"):
    if os.path.isdir(_p) and _p not in sys.path:
        sys.path.append(_p)

import concourse.bacc as bacc
import concourse.bass as bass
import concourse.mybir as mybir
import concourse.tile as tile
from concourse.bass_utils import run_bass_kernel_spmd

ANCHORS = np.array([[10.0, 13.0], [16.0, 30.0], [33.0, 23.0]], dtype=np.float32)
NB_FULL = 16
N_CORES = 8
NB = NB_FULL // N_CORES  # batches per core
NA = 3
NC = 85  # 5 + 80 channels
NG = 76
NPOS = NG * NG  # 5776
STRIDE = 8.0

# Position-chunking: output partition p holds rows [45p, 45p+45); chunk j
# gathers positions {45p + j}. 5776 = 128*45 + 16 -> 16-row tail.
RPP = 45  # rows per partition (main part)
MAIN = 128 * RPP  # 5760
TAIL = NPOS - MAIN  # 16

BASE_B = 128 - NC  # 43: odd pairs put channel c at partition 43+c

F32 = mybir.dt.float32
AF = mybir.ActivationFunctionType
OP = mybir.AluOpType


def _tables():
    p = np.arange(128)[:, None]
    j = np.arange(RPP)[None, :]
    r = p * RPP + j
    gg = np.empty((128, 2 * RPP), dtype=np.float32)
    gg[:, 0::2] = (r % NG) * STRIDE
    gg[:, 1::2] = (r // NG) * STRIDE
    rt = MAIN + np.arange(TAIL)
    gxt = ((rt % NG) * STRIDE).astype(np.float32)[:, None]
    gyt = float((MAIN // NG) * STRIDE)  # rows 5760..5775 all have gy=75
    assert np.all(rt // NG == MAIN // NG)
    # full 128x128 permutation selectors (transpose-mode requires a true
    # permutation): A = identity (even pairs, channels at rows 0..84);
    # B maps row 43+c -> col c, junk rows 0..42 -> junk cols 85..127.
    perm = np.zeros((128, 256), dtype=np.float32)
    perm[np.arange(128), np.arange(128)] = 1.0
    permB = perm[:, 128:]
    permB[BASE_B + np.arange(NC), np.arange(NC)] = 1.0
    permB[np.arange(BASE_B), NC + np.arange(BASE_B)] = 1.0
    return gg, gxt, gyt, perm


GG_TABLE, GXT_TABLE, GYT_CONST, PERM_TABLE = _tables()


def build_program():
    nc = bacc.Bacc(None, target_bir_lowering=False)

    x = nc.dram_tensor("x", (NB, NA * NC, NG, NG), F32, kind="ExternalInput")
    out = nc.dram_tensor("out", (NB, NA * NPOS, NC), F32, kind="ExternalOutput")
    gg = nc.dram_tensor("gg", (128, 2 * RPP), F32, kind="ExternalInput")
    gxt = nc.dram_tensor("gxt", (TAIL, 1), F32, kind="ExternalInput")
    perm = nc.dram_tensor("perm", (128, 256), F32, kind="ExternalInput")

    with tile.TileContext(nc) as tc:
        with (
            tc.tile_pool(name="constp", bufs=1) as constp,
            tc.tile_pool(name="xp", bufs=1) as xp,
            tc.tile_pool(name="outp", bufs=3) as outp,
            tc.tile_pool(name="pp", bufs=3, space="PSUM") as pp,
            tc.tile_pool(name="tp", bufs=2, space="PSUM") as tp,
        ):
            perms = constp.tile([128, 256], F32)
            nc.sync.dma_start(out=perms[:], in_=perm[:])
            ggs = constp.tile([128, 2 * RPP], F32)
            nc.sync.dma_start(out=ggs[:], in_=gg[:])
            gxts = constp.tile([TAIL, 1], F32)
            nc.sync.dma_start(out=gxts[:], in_=gxt[:])
            ggv = ggs.rearrange("p (k c) -> p k c", c=2)

            # manually rotated input tiles, fully memset once: junk rows
            # feed zero selector columns, but must be finite (NaN*0=NaN)
            xts = [
                xp.tile([128, NPOS], F32, name=f"xt{i}", tag=f"xt{i}")
                for i in range(3)
            ]
            for t in xts:
                nc.gpsimd.memset(t[:], 0.0)

            for pair in range(NB * NA):
                b, a = divmod(pair, NA)
                aw = float(ANCHORS[a, 0])
                ah = float(ANCHORS[a, 1])
                base = 0 if pair % 2 == 0 else BASE_B
                sel = perms[:, 128:256] if base else perms[:, 0:128]
                xt = xts[pair % 3]
                nc.gpsimd.dma_start(
                    out=xt[base : base + NC, :],
                    in_=x[b, NC * a : NC * (a + 1)].rearrange("c h w -> c (h w)"),
                )
                ot = outp.tile([128, 3840], F32, tag="ot")
                tt = outp.tile([TAIL, 96], F32, tag="tt")
                # (128, 45, 128): [:, j, :] = chunk j (stride-45 positions)
                xmain = xt[:, 0:MAIN].rearrange("c (m j) -> c j m", j=RPP)

                # main chunks (128 psum cols each; 4 per bank exactly) in
                # groups of 8 sharing a 2-bank PSUM tile
                for k0, nk in ((0, 8), (8, 8), (16, 8), (24, 8), (32, 8), (40, 5)):
                    ps = pp.tile([128, 1024], F32, tag="ps")
                    for m in range(nk):
                        nc.tensor.transpose(
                            ps[:, 128 * m : 128 * m + 128], xmain[:, k0 + m, :], sel
                        )
                    psv = ps[:, 0 : 128 * nk].rearrange("p (k c) -> p k c", c=128)
                    otv = ot[:, k0 * NC : (k0 + nk) * NC].rearrange(
                        "p (k c) -> p k c", c=NC
                    )
                    # evacuate with fused tanh(v/2), then true exp on the
                    # w/h cols straight from PSUM raw values
                    nc.scalar.activation(otv, psv[:, :, 0:NC], AF.Tanh, scale=0.5)
                    nc.scalar.activation(otv[:, :, 2:4], psv[:, :, 2:4], AF.Exp)

                # tail: positions 5760..5775
                pst = tp.tile([TAIL, 512], F32, tag="pst")
                nc.tensor.transpose(pst[:, 0:128], xt[:, MAIN:NPOS], sel)
                nc.scalar.activation(tt[:, 0:NC], pst[:, 0:NC], AF.Tanh, scale=0.5)
                nc.scalar.activation(tt[:, 2:4], pst[:, 2:4], AF.Exp)

                # VectorE fixups (main): whole-tile affine at 2x port mode
                # (needs an even element count -> one memset pad column),
                # then per-channel-type corrections.
                nc.vector.memset(ot[:, 3825:3826], 0.0)
                nc.vector.tensor_scalar(
                    ot[:, 0:3826], ot[:, 0:3826], 0.5, 0.5, OP.mult, OP.add
                )
                otr = ot[:, 0 : RPP * NC].rearrange("p (k c) -> p k c", c=NC)
                xy = otr[:, :, 0:2]
                nc.vector.tensor_scalar(xy, xy, STRIDE, None, OP.mult)
                nc.vector.tensor_tensor(xy, xy, ggv, OP.add)
                wv = otr[:, :, 2:3]
                nc.vector.tensor_scalar(wv, wv, 2.0 * aw, -aw, OP.mult, OP.add)
                hv = otr[:, :, 3:4]
                nc.vector.tensor_scalar(hv, hv, 2.0 * ah, -ah, OP.mult, OP.add)

                # VectorE fixups (tail)
                nc.vector.memset(tt[:, 85:86], 0.0)
                nc.vector.tensor_scalar(
                    tt[:, 0:86], tt[:, 0:86], 0.5, 0.5, OP.mult, OP.add
                )
                nc.vector.tensor_scalar(
                    tt[:, 0:1], tt[:, 0:1], STRIDE, gxts[:], OP.mult, OP.add
                )
                nc.vector.tensor_scalar(
                    tt[:, 1:2], tt[:, 1:2], STRIDE, GYT_CONST, OP.mult, OP.add
                )
                nc.vector.tensor_scalar(
                    tt[:, 2:3], tt[:, 2:3], 2.0 * aw, -aw, OP.mult, OP.add
                )
                nc.vector.tensor_scalar(
                    tt[:, 3:4], tt[:, 3:4], 2.0 * ah, -ah, OP.mult, OP.add
                )

                # stores on the SWDGE queue (GpSimd is otherwise idle)
                obase = a * NPOS
                nc.gpsimd.dma_start(
                    out=out[b, obase : obase + MAIN, :].rearrange(
                        "(p j) c -> p (j c)", p=128
                    ),
                    in_=ot[:, 0 : RPP * NC],
                )
                nc.gpsimd.dma_start(
                    out=out[b, obase + MAIN : obase + NPOS, :], in_=tt[:, 0:NC]
                )

    nc.compile()
    return nc


_NC_CACHE = None


def _get_program():
    global _NC_CACHE
    if _NC_CACHE is None:
        _NC_CACHE = build_program()
    return _NC_CACHE


def run(x, trace=False, **kwargs):
    """x: full (16, 255, 76, 76) f32. Returns (full_out, BassKernelResults)."""
    x = np.ascontiguousarray(np.asarray(x, dtype=np.float32))
    assert x.shape == (NB_FULL, NA * NC, NG, NG), x.shape
    nc = _get_program()
    in_maps = [
        {
            "x": np.ascontiguousarray(x[c * NB : (c + 1) * NB]),
            "gg": GG_TABLE,
            "gxt": GXT_TABLE,
            "perm": PERM_TABLE,
        }
        for c in range(N_CORES)
    ]
    res = run_bass_kernel_spmd(nc, in_maps, list(range(N_CORES)), trace=trace, **kwargs)
    out = np.concatenate([res.results[c]["out"] for c in range(N_CORES)], axis=0)
    return out, res


def kernel(x):
    out, _ = run(x, trace=False)
    return out


# revision 13
# speedup vs baseline: 1.3675x; 1.3675x over previous
"""Trainium2 Bass kernel for YOLO-style DetectionLayer decode.

Full input  x: (16, 255, 76, 76) f32  (channel-major: 3 anchors x 85 ch)
Full output  : (16, 17328, 85) f32   (position-major: 3*76*76 rows x 85 ch)

Math per (b, a, gy, gx):
  out[..., 0] = (sigmoid(tx) + gx) * 8
  out[..., 1] = (sigmoid(ty) + gy) * 8
  out[..., 2] = exp(tw) * ANCHOR[a][0]        (stride cancels)
  out[..., 3] = exp(th) * ANCHOR[a][1]
  out[..., 4:] = sigmoid(...)

Sharding: pure data-parallel over batch: 2 batches per core x 8 cores.

Per-core kernel (per (b, a) pair, 6 pairs):
  - Input block (85 ch, 5776 pos) loads with channels on partitions.
    SBUF ports interleave mod 64 (port 2j serves partitions {4j..4j+3,
    32+4j..35+4j}; odd ports the same at +64), so a fixed 0..84 layout
    piles 8 DMA packets on every even port and <=4 on odd ports.  Pairs
    alternate between partitions 0..84 and 43..127 so overlapping input
    DMAs of adjacent pairs hit complementary ports (max 6/pair).
  - TensorE transposes 46 chunks of (128 part, 128 pos) via one-hot
    selector matrices (I85 at rows 0..84 / rows 43..127; junk rows hit
    zero columns) -> PSUM (128 pos, 85 ch).  Chunk j takes positions
    {45 p + j} so output partition p holds 45 consecutive output rows
    -> 15.3KB contiguous output DMA runs.
  - ScalarE evacuates PSUM with fused tanh(v/2) (sigmoid = .5+.5*tanh;
    one ACT table set has both tanh and exp), plus true Exp on the w/h
    cols straight from PSUM raw values.
  - VectorE: whole-tile affine .5*t+.5 (2x port mode) turns tanh into
    sigmoid; x/y = 8*s + 8*grid (host table); w/h = (2A)*v - A
    (compensates the affine on the exp'd cols).
  - Output DMA per pair on the GpSimd SWDGE queue (sync HWDGE carries
    the input loads; separate rings, and GpSimd is otherwise idle).
"""

import os
import sys

import numpy as np

for _p in ("/opt/trn_rl_repo", "/root/.axon_site/_ro/trn_rl_repo"):
    if os.path.isdir(_p) and _p not in sys.path:
        sys.path.append(_p)

import concourse.bacc as bacc
import concourse.bass as bass
import concourse.mybir as mybir
import concourse.tile as tile
from concourse.bass_utils import run_bass_kernel_spmd

ANCHORS = np.array([[10.0, 13.0], [16.0, 30.0], [33.0, 23.0]], dtype=np.float32)
NB_FULL = 16
N_CORES = 8
NB = NB_FULL // N_CORES  # batches per core
NA = 3
NC = 85  # 5 + 80 channels
NG = 76
NPOS = NG * NG  # 5776
STRIDE = 8.0

# Position-chunking: output partition p holds rows [45p, 45p+45); chunk j
# gathers positions {45p + j}. 5776 = 128*45 + 16 -> 16-row tail.
RPP = 45  # rows per partition (main part)
MAIN = 128 * RPP  # 5760
TAIL = NPOS - MAIN  # 16

BASE_B = 128 - NC  # 43: odd pairs put channel c at partition 43+c

F32 = mybir.dt.float32
AF = mybir.ActivationFunctionType
OP = mybir.AluOpType


def _tables():
    p = np.arange(128)[:, None]
    j = np.arange(RPP)[None, :]
    r = p * RPP + j
    gg = np.empty((128, 2 * RPP), dtype=np.float32)
    gg[:, 0::2] = (r % NG) * STRIDE
    gg[:, 1::2] = (r // NG) * STRIDE
    rt = MAIN + np.arange(TAIL)
    gxt = ((rt % NG) * STRIDE).astype(np.float32)[:, None]
    gyt = float((MAIN // NG) * STRIDE)  # rows 5760..5775 all have gy=75
    assert np.all(rt // NG == MAIN // NG)
    # full 128x128 permutation selectors (transpose-mode requires a true
    # permutation): A = identity (even pairs, channels at rows 0..84);
    # B maps row 43+c -> col c, junk rows 0..42 -> junk cols 85..127.
    perm = np.zeros((128, 256), dtype=np.float32)
    perm[np.arange(128), np.arange(128)] = 1.0
    permB = perm[:, 128:]
    permB[BASE_B + np.arange(NC), np.arange(NC)] = 1.0
    permB[np.arange(BASE_B), NC + np.arange(BASE_B)] = 1.0
    return gg, gxt, gyt, perm


GG_TABLE, GXT_TABLE, GYT_CONST, PERM_TABLE = _tables()


def build_program():
    nc = bacc.Bacc(None, target_bir_lowering=False)

    x = nc.dram_tensor("x", (NB, NA * NC, NG, NG), F32, kind="ExternalInput")
    out = nc.dram_tensor("out", (NB, NA * NPOS, NC), F32, kind="ExternalOutput")
    gg = nc.dram_tensor("gg", (128, 2 * RPP), F32, kind="ExternalInput")
    gxt = nc.dram_tensor("gxt", (TAIL, 1), F32, kind="ExternalInput")
    perm = nc.dram_tensor("perm", (128, 256), F32, kind="ExternalInput")

    with tile.TileContext(nc) as tc:
        with (
            tc.tile_pool(name="constp", bufs=1) as constp,
            tc.tile_pool(name="xp", bufs=1) as xp,
            tc.tile_pool(name="outp", bufs=3) as outp,
            tc.tile_pool(name="pp", bufs=3, space="PSUM") as pp,
            tc.tile_pool(name="tp", bufs=2, space="PSUM") as tp,
        ):
            perms = constp.tile([128, 256], F32)
            nc.sync.dma_start(out=perms[:], in_=perm[:])
            ggs = constp.tile([128, 2 * RPP], F32)
            nc.sync.dma_start(out=ggs[:], in_=gg[:])
            gxts = constp.tile([TAIL, 1], F32)
            nc.sync.dma_start(out=gxts[:], in_=gxt[:])
            ggv = ggs.rearrange("p (k c) -> p k c", c=2)

            # one resident input tile per pair: all six input DMAs issue
            # up-front on the GpSimd SWDGE queue so no output DMA's
            # semaphore wait can block them (in-order issue engine), and
            # alternating partition bases interleave on the SBUF ports.
            # Junk rows feed zero selector columns but must be finite
            # (NaN*0=NaN), so DVE memsets them once.
            xts = []
            for pair in range(NB * NA):
                t = xp.tile([128, NPOS], F32, name=f"xt{pair}", tag=f"xt{pair}")
                xts.append(t)
                # aligned start partitions; the input DMA (emitted after)
                # overwrites the overlapping rows
                if pair % 2 == 0:
                    nc.vector.memset(t[64:128, :], 0.0)
                else:
                    nc.vector.memset(t[0:64, :], 0.0)
            for pair in range(NB * NA):
                b, a = divmod(pair, NA)
                base = 0 if pair % 2 == 0 else BASE_B
                nc.gpsimd.dma_start(
                    out=xts[pair][base : base + NC, :],
                    in_=x[b, NC * a : NC * (a + 1)].rearrange("c h w -> c (h w)"),
                )

            for pair in range(NB * NA):
                b, a = divmod(pair, NA)
                aw = float(ANCHORS[a, 0])
                ah = float(ANCHORS[a, 1])
                base = 0 if pair % 2 == 0 else BASE_B
                sel = perms[:, 128:256] if base else perms[:, 0:128]
                xt = xts[pair]
                ot = outp.tile([128, 3840], F32, tag="ot")
                tt = outp.tile([TAIL, 96], F32, tag="tt")
                # (128, 45, 128): [:, j, :] = chunk j (stride-45 positions)
                xmain = xt[:, 0:MAIN].rearrange("c (m j) -> c j m", j=RPP)

                # main chunks (128 psum cols each; 4 per bank exactly) in
                # groups of 8 sharing a 2-bank PSUM tile
                for k0, nk in ((0, 8), (8, 8), (16, 8), (24, 8), (32, 8), (40, 5)):
                    ps = pp.tile([128, 1024], F32, tag="ps")
                    for m in range(nk):
                        nc.tensor.transpose(
                            ps[:, 128 * m : 128 * m + 128], xmain[:, k0 + m, :], sel
                        )
                    psv = ps[:, 0 : 128 * nk].rearrange("p (k c) -> p k c", c=128)
                    otv = ot[:, k0 * NC : (k0 + nk) * NC].rearrange(
                        "p (k c) -> p k c", c=NC
                    )
                    # evacuate with fused tanh(v/2), then true exp on the
                    # w/h cols straight from PSUM raw values
                    nc.scalar.activation(otv, psv[:, :, 0:NC], AF.Tanh, scale=0.5)
                    nc.scalar.activation(otv[:, :, 2:4], psv[:, :, 2:4], AF.Exp)

                # tail: positions 5760..5775
                pst = tp.tile([TAIL, 512], F32, tag="pst")
                nc.tensor.transpose(pst[:, 0:128], xt[:, MAIN:NPOS], sel)
                nc.scalar.activation(tt[:, 0:NC], pst[:, 0:NC], AF.Tanh, scale=0.5)
                nc.scalar.activation(tt[:, 2:4], pst[:, 2:4], AF.Exp)

                # VectorE fixups (main): whole-tile affine at 2x port mode
                # (needs an even element count -> one memset pad column),
                # then per-channel-type corrections.
                nc.vector.memset(ot[:, 3825:3826], 0.0)
                nc.vector.tensor_scalar(
                    ot[:, 0:3826], ot[:, 0:3826], 0.5, 0.5, OP.mult, OP.add
                )
                otr = ot[:, 0 : RPP * NC].rearrange("p (k c) -> p k c", c=NC)
                xy = otr[:, :, 0:2]
                nc.vector.tensor_scalar(xy, xy, STRIDE, None, OP.mult)
                nc.vector.tensor_tensor(xy, xy, ggv, OP.add)
                wv = otr[:, :, 2:3]
                nc.vector.tensor_scalar(wv, wv, 2.0 * aw, -aw, OP.mult, OP.add)
                hv = otr[:, :, 3:4]
                nc.vector.tensor_scalar(hv, hv, 2.0 * ah, -ah, OP.mult, OP.add)

                # VectorE fixups (tail)
                nc.vector.memset(tt[:, 85:86], 0.0)
                nc.vector.tensor_scalar(
                    tt[:, 0:86], tt[:, 0:86], 0.5, 0.5, OP.mult, OP.add
                )
                nc.vector.tensor_scalar(
                    tt[:, 0:1], tt[:, 0:1], STRIDE, gxts[:], OP.mult, OP.add
                )
                nc.vector.tensor_scalar(
                    tt[:, 1:2], tt[:, 1:2], STRIDE, GYT_CONST, OP.mult, OP.add
                )
                nc.vector.tensor_scalar(
                    tt[:, 2:3], tt[:, 2:3], 2.0 * aw, -aw, OP.mult, OP.add
                )
                nc.vector.tensor_scalar(
                    tt[:, 3:4], tt[:, 3:4], 2.0 * ah, -ah, OP.mult, OP.add
                )

                # stores on the SWDGE queue (GpSimd is otherwise idle)
                obase = a * NPOS
                nc.gpsimd.dma_start(
                    out=out[b, obase : obase + MAIN, :].rearrange(
                        "(p j) c -> p (j c)", p=128
                    ),
                    in_=ot[:, 0 : RPP * NC],
                )
                nc.gpsimd.dma_start(
                    out=out[b, obase + MAIN : obase + NPOS, :], in_=tt[:, 0:NC]
                )

    nc.compile()
    return nc


_NC_CACHE = None


def _get_program():
    global _NC_CACHE
    if _NC_CACHE is None:
        _NC_CACHE = build_program()
    return _NC_CACHE


def run(x, trace=False, **kwargs):
    """x: full (16, 255, 76, 76) f32. Returns (full_out, BassKernelResults)."""
    x = np.ascontiguousarray(np.asarray(x, dtype=np.float32))
    assert x.shape == (NB_FULL, NA * NC, NG, NG), x.shape
    nc = _get_program()
    in_maps = [
        {
            "x": np.ascontiguousarray(x[c * NB : (c + 1) * NB]),
            "gg": GG_TABLE,
            "gxt": GXT_TABLE,
            "perm": PERM_TABLE,
        }
        for c in range(N_CORES)
    ]
    res = run_bass_kernel_spmd(nc, in_maps, list(range(N_CORES)), trace=trace, **kwargs)
    out = np.concatenate([res.results[c]["out"] for c in range(N_CORES)], axis=0)
    return out, res


def kernel(x):
    out, _ = run(x, trace=False)
    return out


# revision 17
# speedup vs baseline: 1.6322x; 1.1936x over previous
"""Trainium2 Bass kernel for YOLO-style DetectionLayer decode.

Full input  x: (16, 255, 76, 76) f32  (channel-major: 3 anchors x 85 ch)
Full output  : (16, 17328, 85) f32   (position-major: 3*76*76 rows x 85 ch)

Math per (b, a, gy, gx):
  out[..., 0] = (sigmoid(tx) + gx) * 8
  out[..., 1] = (sigmoid(ty) + gy) * 8
  out[..., 2] = exp(tw) * ANCHOR[a][0]        (stride cancels)
  out[..., 3] = exp(th) * ANCHOR[a][1]
  out[..., 4:] = sigmoid(...)

Sharding: pure data-parallel over batch: 2 batches per core x 8 cores.

Per-core kernel (per (b, a) pair, 6 pairs):
  - Input block (85 ch, 5776 pos) loads with channels on partitions.
    SBUF ports interleave mod 64 (port 2j serves partitions {4j..4j+3,
    32+4j..35+4j}; odd ports the same at +64), so a fixed 0..84 layout
    piles 8 DMA packets on every even port and <=4 on odd ports.  Pairs
    alternate between partitions 0..84 and 43..127 so overlapping input
    DMAs of adjacent pairs hit complementary ports (max 6/pair).
  - TensorE transposes 46 chunks of (128 part, 128 pos) via one-hot
    selector matrices (I85 at rows 0..84 / rows 43..127; junk rows hit
    zero columns) -> PSUM (128 pos, 85 ch).  Chunk j takes positions
    {45 p + j} so output partition p holds 45 consecutive output rows
    -> 15.3KB contiguous output DMA runs.
  - ScalarE evacuates PSUM with fused tanh(v/2) (sigmoid = .5+.5*tanh;
    one ACT table set has both tanh and exp), plus true Exp on the w/h
    cols straight from PSUM raw values.
  - VectorE: whole-tile affine .5*t+.5 (2x port mode) turns tanh into
    sigmoid; x/y = 8*s + 8*grid (host table); w/h = (2A)*v - A
    (compensates the affine on the exp'd cols).
  - Output DMA per pair on the GpSimd SWDGE queue (sync HWDGE carries
    the input loads; separate rings, and GpSimd is otherwise idle).
"""

import os
import sys

import numpy as np

for _p in ("/opt/trn_rl_repo", "/root/.axon_site/_ro/trn_rl_repo"):
    if os.path.isdir(_p) and _p not in sys.path:
        sys.path.append(_p)

import concourse.bacc as bacc
import concourse.bass as bass
import concourse.mybir as mybir
import concourse.tile as tile
from concourse.bass_utils import run_bass_kernel_spmd

ANCHORS = np.array([[10.0, 13.0], [16.0, 30.0], [33.0, 23.0]], dtype=np.float32)
NB_FULL = 16
N_CORES = 8
NB = NB_FULL // N_CORES  # batches per core
NA = 3
NC = 85  # 5 + 80 channels
NG = 76
NPOS = NG * NG  # 5776
STRIDE = 8.0

# Position-chunking: output partition p holds rows [45p, 45p+45); chunk j
# gathers positions {45p + j}. 5776 = 128*45 + 16 -> 16-row tail.
RPP = 45  # rows per partition (main part)
MAIN = 128 * RPP  # 5760
TAIL = NPOS - MAIN  # 16

BASE_B = 128 - NC  # 43: odd pairs put channel c at partition 43+c

F32 = mybir.dt.float32
AF = mybir.ActivationFunctionType
OP = mybir.AluOpType


def _tables():
    p = np.arange(128)[:, None]
    j = np.arange(RPP)[None, :]
    r = p * RPP + j
    gg = np.empty((128, 2 * RPP), dtype=np.float32)
    gg[:, 0::2] = (r % NG) * STRIDE
    gg[:, 1::2] = (r // NG) * STRIDE
    rt = MAIN + np.arange(TAIL)
    gxt = ((rt % NG) * STRIDE).astype(np.float32)[:, None]
    gyt = float((MAIN // NG) * STRIDE)  # rows 5760..5775 all have gy=75
    assert np.all(rt // NG == MAIN // NG)
    # full 128x128 permutation selectors (transpose-mode requires a true
    # permutation): A = identity (even pairs, channels at rows 0..84);
    # B maps row 43+c -> col c, junk rows 0..42 -> junk cols 85..127.
    perm = np.zeros((128, 256), dtype=np.float32)
    perm[np.arange(128), np.arange(128)] = 1.0
    permB = perm[:, 128:]
    permB[BASE_B + np.arange(NC), np.arange(NC)] = 1.0
    permB[np.arange(BASE_B), NC + np.arange(BASE_B)] = 1.0
    return gg, gxt, gyt, perm


GG_TABLE, GXT_TABLE, GYT_CONST, PERM_TABLE = _tables()


def build_program():
    nc = bacc.Bacc(None, target_bir_lowering=False)

    x = nc.dram_tensor("x", (NB, NA * NC, NG, NG), F32, kind="ExternalInput")
    out = nc.dram_tensor("out", (NB, NA * NPOS, NC), F32, kind="ExternalOutput")
    gg = nc.dram_tensor("gg", (128, 2 * RPP), F32, kind="ExternalInput")
    gxt = nc.dram_tensor("gxt", (TAIL, 1), F32, kind="ExternalInput")
    perm = nc.dram_tensor("perm", (128, 256), F32, kind="ExternalInput")

    with tile.TileContext(nc) as tc:
        with (
            tc.tile_pool(name="constp", bufs=1) as constp,
            tc.tile_pool(name="xp", bufs=1) as xp,
            tc.tile_pool(name="outp", bufs=3) as outp,
            tc.tile_pool(name="pp", bufs=3, space="PSUM") as pp,
            tc.tile_pool(name="tp", bufs=2, space="PSUM") as tp,
        ):
            perms = constp.tile([128, 256], F32)
            nc.sync.dma_start(out=perms[:], in_=perm[:])
            ggs = constp.tile([128, 2 * RPP], F32)
            nc.sync.dma_start(out=ggs[:], in_=gg[:])
            gxts = constp.tile([TAIL, 1], F32)
            nc.sync.dma_start(out=gxts[:], in_=gxt[:])
            ggv = ggs.rearrange("p (k c) -> p k c", c=2)

            # Four input tiles, two per partition-parity flavor, reused
            # round-robin within a flavor.  Junk rows feed zero selector
            # columns but must be finite (NaN*0=NaN): they're memset ONCE
            # and never rewritten — the input DMA loads 96 rows (85 real
            # channels + 11 junk neighbors from the flat channel stream,
            # always in-bounds) so it never touches the memset region and
            # carries no dependency on it.  All six input DMAs issue
            # up-front on the GpSimd SWDGE queue so no output DMA's
            # semaphore wait can block them (in-order issue engine), and
            # alternating partition bases interleave on the SBUF ports.
            xf = x.rearrange("b c h w -> (b c) (h w)")
            xtE = [xp.tile([128, NPOS], F32, name=f"xtE{i}", tag=f"xtE{i}") for i in range(2)]
            xtO = [xp.tile([128, NPOS], F32, name=f"xtO{i}", tag=f"xtO{i}") for i in range(2)]
            for t in xtE:
                nc.vector.memset(t[96:128, :], 0.0)
            for t in xtO:
                nc.vector.memset(t[0:32, :], 0.0)

            def xt_of(pair):
                return xtE[(pair // 2) % 2] if pair % 2 == 0 else xtO[(pair // 2) % 2]

            def load_pair(pair):
                b, a = divmod(pair, NA)
                s = (b * NA + a) * NC
                if pair % 2 == 0:
                    nc.gpsimd.dma_start(
                        out=xt_of(pair)[0:96, :], in_=xf[s : s + 96, :]
                    )
                else:
                    nc.gpsimd.dma_start(
                        out=xt_of(pair)[32:128, :], in_=xf[s - 11 : s + NC, :]
                    )

            # first 4 loads up-front; loads 4/5 are emitted after pair 0/1's
            # transposes so the WAR reuse dependency is tracked correctly
            for pair in range(4):
                load_pair(pair)

            for pair in range(NB * NA):
                b, a = divmod(pair, NA)
                aw = float(ANCHORS[a, 0])
                ah = float(ANCHORS[a, 1])
                base = 0 if pair % 2 == 0 else BASE_B
                sel = perms[:, 128:256] if base else perms[:, 0:128]
                xt = xt_of(pair)
                ot = outp.tile([128, 3840], F32, tag="ot")
                tt = outp.tile([TAIL, 96], F32, tag="tt")
                # (128, 45, 128): [:, j, :] = chunk j (stride-45 positions)
                xmain = xt[:, 0:MAIN].rearrange("c (m j) -> c j m", j=RPP)

                # main chunks (128 psum cols each; 4 per bank exactly) in
                # groups of 8 sharing a 2-bank PSUM tile
                for k0, nk in ((0, 8), (8, 8), (16, 8), (24, 8), (32, 8), (40, 5)):
                    ps = pp.tile([128, 1024], F32, tag="ps")
                    for m in range(nk):
                        nc.tensor.transpose(
                            ps[:, 128 * m : 128 * m + 128], xmain[:, k0 + m, :], sel
                        )
                    psv = ps[:, 0 : 128 * nk].rearrange("p (k c) -> p k c", c=128)
                    otv = ot[:, k0 * NC : (k0 + nk) * NC].rearrange(
                        "p (k c) -> p k c", c=NC
                    )
                    # evacuate with fused tanh(v/2), then true exp on the
                    # w/h cols straight from PSUM raw values
                    nc.scalar.activation(otv, psv[:, :, 0:NC], AF.Tanh, scale=0.5)
                    nc.scalar.activation(otv[:, :, 2:4], psv[:, :, 2:4], AF.Exp)

                # tail: positions 5760..5775
                pst = tp.tile([TAIL, 512], F32, tag="pst")
                nc.tensor.transpose(pst[:, 0:128], xt[:, MAIN:NPOS], sel)
                # this pair's reads of its input tile are all emitted; the
                # deferred reload of the shared tile can now be tracked
                if pair + 4 < NB * NA:
                    load_pair(pair + 4)
                nc.scalar.activation(tt[:, 0:NC], pst[:, 0:NC], AF.Tanh, scale=0.5)
                nc.scalar.activation(tt[:, 2:4], pst[:, 2:4], AF.Exp)

                # VectorE fixups (main): whole-tile affine at 2x port mode
                # (needs an even element count -> one memset pad column),
                # then per-channel-type corrections.
                nc.vector.memset(ot[:, 3825:3826], 0.0)
                nc.vector.tensor_scalar(
                    ot[:, 0:3826], ot[:, 0:3826], 0.5, 0.5, OP.mult, OP.add
                )
                otr = ot[:, 0 : RPP * NC].rearrange("p (k c) -> p k c", c=NC)
                xy = otr[:, :, 0:2]
                nc.vector.tensor_scalar(xy, xy, STRIDE, None, OP.mult)
                nc.vector.tensor_tensor(xy, xy, ggv, OP.add)
                wv = otr[:, :, 2:3]
                nc.vector.tensor_scalar(wv, wv, 2.0 * aw, -aw, OP.mult, OP.add)
                hv = otr[:, :, 3:4]
                nc.vector.tensor_scalar(hv, hv, 2.0 * ah, -ah, OP.mult, OP.add)

                # VectorE fixups (tail)
                nc.vector.memset(tt[:, 85:86], 0.0)
                nc.vector.tensor_scalar(
                    tt[:, 0:86], tt[:, 0:86], 0.5, 0.5, OP.mult, OP.add
                )
                nc.vector.tensor_scalar(
                    tt[:, 0:1], tt[:, 0:1], STRIDE, gxts[:], OP.mult, OP.add
                )
                nc.vector.tensor_scalar(
                    tt[:, 1:2], tt[:, 1:2], STRIDE, GYT_CONST, OP.mult, OP.add
                )
                nc.vector.tensor_scalar(
                    tt[:, 2:3], tt[:, 2:3], 2.0 * aw, -aw, OP.mult, OP.add
                )
                nc.vector.tensor_scalar(
                    tt[:, 3:4], tt[:, 3:4], 2.0 * ah, -ah, OP.mult, OP.add
                )

                # stores on the SWDGE queue (GpSimd is otherwise idle)
                obase = a * NPOS
                nc.gpsimd.dma_start(
                    out=out[b, obase : obase + MAIN, :].rearrange(
                        "(p j) c -> p (j c)", p=128
                    ),
                    in_=ot[:, 0 : RPP * NC],
                )
                nc.gpsimd.dma_start(
                    out=out[b, obase + MAIN : obase + NPOS, :], in_=tt[:, 0:NC]
                )

    nc.compile()
    return nc


_NC_CACHE = None


def _get_program():
    global _NC_CACHE
    if _NC_CACHE is None:
        _NC_CACHE = build_program()
    return _NC_CACHE


def run(x, trace=False, **kwargs):
    """x: full (16, 255, 76, 76) f32. Returns (full_out, BassKernelResults)."""
    x = np.ascontiguousarray(np.asarray(x, dtype=np.float32))
    assert x.shape == (NB_FULL, NA * NC, NG, NG), x.shape
    nc = _get_program()
    in_maps = [
        {
            "x": np.ascontiguousarray(x[c * NB : (c + 1) * NB]),
            "gg": GG_TABLE,
            "gxt": GXT_TABLE,
            "perm": PERM_TABLE,
        }
        for c in range(N_CORES)
    ]
    res = run_bass_kernel_spmd(nc, in_maps, list(range(N_CORES)), trace=trace, **kwargs)
    out = np.concatenate([res.results[c]["out"] for c in range(N_CORES)], axis=0)
    return out, res


def kernel(x):
    out, _ = run(x, trace=False)
    return out
